# revision 16
# baseline (speedup 1.0000x reference)
"""Single-head attention on 8 Trainium2 NeuronCores (v2).

Problem: x[4,4096,1024] @ {Wq,Wk,Wv}[1024,64] -> scaled-dot-product
attention per batch -> out[4,4096,64].

Sharding: core c handles batch b=c//2, query half h=c%2 (2048 queries),
with K/V over the full 4096-key sequence of its batch. No collectives.

Host prep (free): per core, x is permuted (own query half first), cast
to bf16, and laid out piece-major as xT[8 pieces][128 part][8 ech x 512]
so every load DMA is a fully contiguous 1MB block (8KB per partition).

Dataflow per core:
  QT[128,2048] = [Wq|Wq]^T xT (own half)  (dup rows so scores rhs can
       sit at partitions 64:128, matching the KT stationary row group)
  VT/KT[128,4096] = [Wv|Wk]^T xT          (VT rows 0:64, KT rows 64:128)
  V'[128,65] per s-tile via DMA-xbar transpose (sync ring) of VT, with a
       ones column for the softmax denominator (no PE transposes).
  Steps (k outer 0..15 over s-tile pairs, th inner 0..3 over 512-query
  blocks): scoresT = KT-tile^T Q (2 MMs, rows 64:127), one exp ACTIVATE
  over [128,1024] (scale 1/8 fused; scores bounded ~|8| so fp32 softmax
  needs no running max), AV = V'^T P (2 MMs accumulating [65,512] in the
  same psum tile after exp consumed it), DVE flush-adds the partial into
  an SBUF accumulator per th block.  PSUM: 3x[128,1024] step tiles +
  2x[128,512] projection-quad tiles = 8 banks.

DMA rings: bulk xT pieces on the Activation HWDGE ring (dispatched
before the exp stream starts), weights + V' transposes on the Sync
ring, output stores on the GpSimd SWDGE ring.  Host divides rows 0:64
by row 64 and transposes back.
"""

import numpy as np

B, T, E, D = 4, 4096, 1024, 64
HALF = T // 2  # queries per core
NCORES = 8

EC = E // 128  # 8 contraction chunks
NPIECE = 8  # 512-query pieces of xT
NS = T // 128  # 32 s-tiles
NKG = NS // 2  # 16 k-groups (2 s-tiles per step)
NTH = HALF // 512  # 4 query blocks
NJUNK = 10  # PE warm-up matmuls during the startup DMA wait

_compiled = {}


def _patch_tile_drain():
    """This walrus build accepts only one sem-wait on the TileContext exit
    drain; spread the waits across preceding nofuse NOPs instead."""
    import concourse.tile as tile
    import concourse.mybir as mybir
    from concourse.tile import ScopedClock

    if getattr(tile.TileContext, "_drain_patch_installed", False):
        return

    def _drain_and_barrier(self, tick_clock, wait_clock):
        nops = [
            self.nc.sync.nop(nofuse=True, hint=f"drain_wait_{i}") for i in range(26)
        ]
        drain_inst = self.nc.sync.drain()
        wait_clock.add_sem_waits(
            drain_inst.ins, ScopedClock({None: tick_clock.global_clock})
        )
        si = drain_inst.ins.sync_info
        if si is not None and len(si.on_wait) > 1:
            waits = list(si.on_wait)
            assert len(waits) - 1 <= len(nops), f"{len(waits)} drain waits"
            si.on_wait = [waits[-1]]
            for w, nop in zip(waits[:-1], nops):
                nop.ins.sync_info = mybir.SyncInfo(on_wait=[w], on_update=[])

        self.nc.all_engine_barrier()
        assert self.sems is not None
        popped = self.nc._tile_sem_poison_stack.pop()
        assert popped is self._sem_poison
        self.nc.clear_and_free_semaphores(list(self.sems.allocated().values()))
        self.nc.all_engine_barrier()

    tile.TileContext._drain_and_barrier = _drain_and_barrier
    tile.TileContext._drain_patch_installed = True


def _dedupe_ldweights(nc):
    """Drop InstLdweights that reload the exact stationary operand the PE
    array already holds (only matmuls in between on the PE queue)."""
    import concourse.mybir as mybir

    for fn in nc.m.functions:
        for blk in fn.blocks:
            new_insts = []
            last_sig = None
            for inst in blk.instructions:
                tn = type(inst).__name__
                if getattr(inst, "engine", None) != mybir.EngineType.PE:
                    new_insts.append(inst)
                    continue
                if tn == "InstLdweights":
                    ap = inst.ins[0]
                    sig = (
                        getattr(ap, "memref", None),
                        getattr(ap, "offset", None),
                        str(getattr(ap, "ap", None)),
                        getattr(ap, "dtype", None),
                    )
                    if sig == last_sig:
                        si = inst.sync_info
                        if si is not None and (si.on_wait or si.on_update):
                            nop = mybir.InstNoOp(
                                name=f"{inst.name}-ldwdup",
                                ins=[],
                                outs=[],
                                bass_is_fusable=False,
                            )
                            nop.engine = inst.engine
                            nop.sync_info = si
                            new_insts.append(nop)
                        continue
                    last_sig = sig
                elif tn != "InstMatmult":
                    last_sig = None
                new_insts.append(inst)
            blk.instructions[:] = new_insts


def _split_multi_waits(nc):
    """This walrus build accepts only one sem-wait per instruction; hoist
    extra waits onto same-engine NoOps inserted just before the owner."""
    import concourse.mybir as mybir

    for fn in nc.m.functions:
        for blk in fn.blocks:
            new_insts = []
            for inst in blk.instructions:
                si = inst.sync_info
                if si is not None and len(si.on_wait) > 1:
                    waits = list(si.on_wait)
                    si.on_wait = [waits[-1]]
                    for j, w in enumerate(waits[:-1]):
                        nop = mybir.InstNoOp(
                            name=f"{inst.name}-waitsplit-{j}",
                            ins=[],
                            outs=[],
                            bass_is_fusable=False,
                        )
                        nop.engine = inst.engine
                        nop.sync_info = mybir.SyncInfo(on_wait=[w], on_update=[])
                        new_insts.append(nop)
                new_insts.append(inst)
            blk.instructions[:] = new_insts


def _build_nc(post=True):
    import os
    import concourse.bass as bass
    import concourse.mybir as mybir
    from concourse.tile import TileContext
    from concourse.masks import make_identity

    vp_mode = os.environ.get("VP_MODE", "pe")

    _patch_tile_drain()

    fp32 = mybir.dt.float32
    bf16 = mybir.dt.bfloat16
    Exp = mybir.ActivationFunctionType.Exp
    ADD = mybir.AluOpType.add
    MULT = mybir.AluOpType.mult

    nc = bass.Bass()

    xT_ext = nc.declare_dram_parameter(
        "xT", [NPIECE, 128, EC * 512], bf16, isOutput=False
    )
    wqq_ext = nc.declare_dram_parameter("w_qq", [128, EC * 128], bf16, isOutput=False)
    wvk_ext = nc.declare_dram_parameter("w_vk", [128, EC * 128], bf16, isOutput=False)
    bqq_ext = nc.declare_dram_parameter("b_qq", [128, 1], fp32, isOutput=False)
    bvk_ext = nc.declare_dram_parameter("b_vk", [128, 1], fp32, isOutput=False)
    out_ext = nc.declare_dram_parameter("outT", [D + 1, HALF], fp32, isOutput=True)

    with TileContext(nc) as tc:
        with (
            tc.tile_pool(name="w", bufs=1) as wpool,
            tc.tile_pool(name="xt", bufs=8) as xtpool,
            tc.tile_pool(name="big", bufs=1) as bigpool,
            tc.tile_pool(name="pt", bufs=3) as ptpool,
            tc.tile_pool(name="ps", bufs=3, space="PSUM") as pspool,
            tc.tile_pool(name="pj", bufs=2, space="PSUM") as pjpool,
        ):
            # --- weights + biases on the gpsimd ring (parallel with the
            # bulk pieces on sync) ---
            wqq_sb = wpool.tile([128, EC * 128], bf16, tag="wqq")
            wvk_sb = wpool.tile([128, EC * 128], bf16, tag="wvk")
            nc.gpsimd.dma_start(out=wqq_sb[:], in_=wqq_ext[:])
            nc.gpsimd.dma_start(out=wvk_sb[:], in_=wvk_ext[:])
            ball_sb = wpool.tile([128, 2], fp32, tag="ball")
            nc.gpsimd.dma_start(out=ball_sb[:, 0:1], in_=bqq_ext[:])
            nc.gpsimd.dma_start(out=ball_sb[:, 1:2], in_=bvk_ext[:])
            bqq_sb = ball_sb[:, 0:1]
            bvk_sb = ball_sb[:, 1:2]

            # --- bulk xT pieces on the Sync HWDGE ring (Scalar stays free
            # for the exp stream), two half dispatches per piece ---
            xts = []
            for j in range(NPIECE):
                xt = xtpool.tile([128, EC * 512], bf16, tag="xt", name=f"xt{j}")
                nc.sync.dma_start(
                    out=xt[:, 0 : EC * 256], in_=xT_ext[j, :, 0 : EC * 256]
                )
                nc.sync.dma_start(
                    out=xt[:, EC * 256 : EC * 512], in_=xT_ext[j, :, EC * 256 :]
                )
                xts.append(xt)

            # --- PE warm-up during the DMA wait + exp table preload.
            # Junk matmuls chained to each arriving piece half keep the
            # HAM activity window alive through the load wait, so the
            # first projection quads run at 2.4 GHz. ---
            jw_sb = wpool.tile([128, 64], bf16, tag="jw")
            nc.vector.memset(jw_sb[:], 0.0)
            tl1 = wpool.tile([128, 1], fp32, tag="tl1")
            nc.scalar.activation(tl1[:], jw_sb[:, 0:1], Exp)
            psj = pjpool.tile([128, 512], fp32, tag="pj", name="psjunk")
            for _ in range(NJUNK):
                nc.tensor.matmul(psj[0:64, 0:64], lhsT=jw_sb[:], rhs=jw_sb[:])
            for j in range(2):
                for h in range(2):
                    for r in range(3):
                        nc.tensor.matmul(
                            psj[0:64, 0:64],
                            lhsT=jw_sb[:],
                            rhs=xts[j][:, h * EC * 256 + r * 64 : h * EC * 256 + r * 64 + 64],
                        )

            qq_sb = bigpool.tile([128, HALF], bf16, tag="qq")
            vk_sb = bigpool.tile([128, T], bf16, tag="vk")
            # V' tiles: [128, 65] per s-tile, ones in column 64
            vp_sb = bigpool.tile([128, NS * 65], bf16, tag="vp")
            nc.vector.memset(vp_sb[:], 1.0)
            if vp_mode == "pe":
                ident = wpool.tile([64, 64], bf16, tag="ident")
                make_identity(nc, ident[:])
            accs = [
                bigpool.tile([D + 1, 512], fp32, tag=f"acc{th}", name=f"acc{th}")
                for th in range(NTH)
            ]

            # --- projection quads: per (piece, q|vk) 8 accumulating
            # matmuls + bias add; vk quads also build 4 V' tiles.  Low
            # scheduler priority so quads fill PE slack instead of
            # front-running the score matmuls and starving the exp
            # stream. ---
            def emit_quad(kind, j):
                with tc.high_priority(offset=-1_000_000):
                    _emit_quad(kind, j)

            def _emit_quad(kind, j):
                pj = pjpool.tile([128, 512], fp32, tag="pj", name=f"pj{kind}{j}")
                w_sb = wqq_sb if kind == "q" else wvk_sb
                xt = xts[j]
                for e in range(EC):
                    nc.tensor.matmul(
                        pj[:],
                        lhsT=w_sb[:, e * 128 : (e + 1) * 128],
                        rhs=xt[:, e * 512 : (e + 1) * 512],
                        start=(e == 0),
                        stop=(e == EC - 1),
                    )
                cols = slice(j * 512, (j + 1) * 512)
                if kind == "q":
                    nc.vector.tensor_scalar(
                        qq_sb[:, cols], pj[:], bqq_sb[:], None, op0=ADD
                    )
                else:
                    nc.vector.tensor_scalar(
                        vk_sb[:, cols], pj[:], bvk_sb[:], None, op0=ADD
                    )
                    for si in range(4 * j, 4 * j + 4):
                        if vp_mode == "dma":
                            nc.sync.dma_start_transpose(
                                out=vp_sb[:, si * 65 : si * 65 + 64],
                                in_=vk_sb[0:64, si * 128 : (si + 1) * 128],
                            )
                        else:
                            pvt = pjpool.tile(
                                [128, 64], bf16, tag="pj", name=f"pvt{si}"
                            )
                            nc.tensor.transpose(
                                pvt[:],
                                vk_sb[0:64, si * 128 : (si + 1) * 128],
                                ident[:],
                            )
                            nc.vector.tensor_copy(
                                out=vp_sb[:, si * 65 : si * 65 + 64], in_=pvt[:]
                            )

            def emit_scores(k, th):
                ps = pspool.tile([128, 1024], fp32, tag="ps", name=f"ps{k}_{th}")
                qcols = slice(th * 512, (th + 1) * 512)
                for loc in range(2):
                    si = 2 * k + loc
                    nc.tensor.matmul(
                        ps[:, loc * 512 : (loc + 1) * 512],
                        lhsT=vk_sb[64:128, si * 128 : (si + 1) * 128],
                        rhs=qq_sb[64:128, qcols],
                    )
                return ps

            def emit_exp(ps):
                pt = ptpool.tile([128, 1024], bf16, tag="pt")
                nc.scalar.activation(pt[:], ps[:], Exp, scale=0.125)
                return pt

            def emit_av_flush(k, th, ps, pt):
                for loc in range(2):
                    si = 2 * k + loc
                    nc.tensor.matmul(
                        ps[0 : D + 1, 0:512],
                        lhsT=vp_sb[:, si * 65 : (si + 1) * 65],
                        rhs=pt[:, loc * 512 : (loc + 1) * 512],
                        start=(loc == 0),
                        stop=(loc == 1),
                    )
                acc = accs[th]
                if k == 0:
                    nc.vector.tensor_copy(out=acc[:], in_=ps[0 : D + 1, 0:512])
                else:
                    nc.vector.scalar_tensor_tensor(
                        acc[:], ps[0 : D + 1, 0:512], 1.0, acc[:], op0=MULT, op1=ADD
                    )
                if k == NKG - 1:
                    nc.gpsimd.dma_start(
                        out=out_ext[:, th * 512 : (th + 1) * 512], in_=acc[:]
                    )

            # quad due-schedule: VK quad j feeds s-tiles 4j..4j+3, first
            # used at step s=8j; Q quad th feeds query block th, first
            # used at step s=th.  Emit each a few steps early.
            quad_sched = {}
            quad_sched.setdefault(0, []).extend(
                [("q", 1, 0), ("q", 2, 0), ("q", 3, 0), ("vk", 1, 0)]
            )
            for j in range(2, NPIECE):
                quad_sched.setdefault(8 * j - 6, []).append(("vk", j, 1))

            _emit_quad("vk", 0)
            _emit_quad("q", 0)

            steps = [(k, th) for k in range(NKG) for th in range(NTH)]
            pend = None
            for s, (k, th) in enumerate(steps):
                ps = emit_scores(k, th)
                for kind, j, lowp in quad_sched.get(s, ()):
                    if lowp:
                        emit_quad(kind, j)
                    else:
                        _emit_quad(kind, j)
                pt = emit_exp(ps)
                if pend is not None:
                    emit_av_flush(*pend)
                pend = (k, th, ps, pt)
            emit_av_flush(*pend)

    nc.finalize()
    if post:
        _dedupe_ldweights(nc)
        _split_multi_waits(nc)
    return nc


def _get_nc():
    if "nc" not in _compiled:
        import os

        post = os.environ.get("KERNEL_NO_POST") != "1"
        _compiled["nc"] = _build_nc(post=post)
    return _compiled["nc"]


def _make_in_maps(x, Wq, bq, Wk, bk, Wv, bv):
    import ml_dtypes

    bf16 = ml_dtypes.bfloat16

    def chunk_major(w2):  # [E, 128] -> [128, EC*128], chunk-major per row
        return np.ascontiguousarray(
            w2.reshape(EC, 128, 128).transpose(1, 0, 2).reshape(128, EC * 128)
        ).astype(bf16)

    w_qq = chunk_major(np.concatenate([Wq, Wq], axis=1))
    w_vk = chunk_major(np.concatenate([Wv, Wk], axis=1))
    b_qq = np.concatenate([bq, bq]).reshape(128, 1).astype(np.float32)
    b_vk = np.concatenate([bv, bk]).reshape(128, 1).astype(np.float32)

    in_maps = []
    for c in range(NCORES):
        b, h = divmod(c, 2)
        xb = x[b]  # [T, E]
        if h == 0:
            xperm = xb
        else:
            xperm = np.concatenate([xb[HALF:], xb[:HALF]], axis=0)
        # piece j, element [p, e*512+t] = xperm[j*512+t, e*128+p]
        xp = np.ascontiguousarray(
            xperm.reshape(NPIECE, 512, EC, 128).transpose(0, 3, 2, 1)
        ).astype(bf16)
        xp = xp.reshape(NPIECE, 128, EC * 512)
        in_maps.append(
            {"xT": xp, "w_qq": w_qq, "w_vk": w_vk, "b_qq": b_qq, "b_vk": b_vk}
        )
    return in_maps


def _assemble(results):
    out = np.empty((B, T, D), np.float32)
    for c in range(NCORES):
        b, h = divmod(c, 2)
        ot = results[c]["outT"]  # [65, HALF]
        out[b, h * HALF : (h + 1) * HALF, :] = (ot[:D] / ot[D : D + 1]).T
    return out


def kernel(x, Wq, bq, Wk, bk, Wv, bv):
    x = np.asarray(x, dtype=np.float32)
    Wq = np.asarray(Wq, dtype=np.float32)
    Wk = np.asarray(Wk, dtype=np.float32)
    Wv = np.asarray(Wv, dtype=np.float32)
    bq = np.asarray(bq, dtype=np.float32)
    bk = np.asarray(bk, dtype=np.float32)
    bv = np.asarray(bv, dtype=np.float32)

    from concourse.bass_utils import run_bass_kernel_spmd

    in_maps = _make_in_maps(x, Wq, bq, Wk, bk, Wv, bv)
    nc = _get_nc()
    res = run_bass_kernel_spmd(nc, in_maps, list(range(NCORES)))
    return _assemble(res.results)
